# revision 47
# baseline (speedup 1.0000x reference)
"""MinGRU kernel for Trainium2 (8 NeuronCores, Bass/Tile) — final.

Measured 82.3-85.1 us exec across runs (baseline 90.5us), rel err 1.38e-3.

Reference computation (B=4, L=8192, D=512, fp32):
    gates = sigmoid(x @ Wg.T + bg)
    cands = tanh(x @ Wc.T + bc)
    h_t   = (1 - g_t) * h_{t-1} + g_t * c_t   (scan along L, h_0 = 0)

Sharding: core c -> (batch b = c//2, channel half = c%2). Each core computes
its batch's full L range for 256 of the 512 output channels; the scan along L
is per (b, channel) so no cross-core communication is needed.

Measured engine budget per core (v1 trace): PE 54.6us (fixed roofline:
131072 PE rows), DVE scan 2.09ns/elem + bneg STT 1.04ns/elem over 16384
elems/lane = 51us payload — DVE is the co-bottleneck, so the design
minimizes DVE instruction count and keeps the dependency graph
single-chain (v2's DVE<->GpSimd ping-pong doubled semaphore costs and
regressed; GpSimd tensor ops run at ~2ns/elem + ~570ns/op and are not
worth it).

v3 vs v1 (90.5us):
  * x and W cast to fp16 on the host: input DMA halves (16.8 -> 8.9 MB/core);
    the x feed (33us queue-wall) ducks well under the PE roofline.
  * Scan units of 2048 tokens: one STT + one scan per (unit, e-tile) with
    matmul/ACT filling the unit in 1024-token halves ([128,1024] fp32 PSUM
    tiles = 2 banks, tags g/c x bufs 2 = 8 banks, still double-buffered).
    Fewer DVE ops -> less fixed overhead and fewer semaphores.
  * Activations read the full 1024-token PSUM tile in one instruction.
  * 26 warm-up matmuls on a zeroed dummy tile while the first weight/x DMAs
    fly: PE_HAM releases the 4/8 cold clock gate before real matmuls start,
    and the PE is never idle long enough to re-throttle.
  * wg ships in two pieces (dc chunk 0 first) so the first real matmul only
    waits for 64KB of weights plus the first x segment.
  * Segment ramp [512, 1024, 1536, 2048, ...] matched to the x queue's
    ~0.37 MB/us delivery rate so the PE rarely outruns the feed.
  * -bg negated on the host; h stored fp16 [2, 128, L] and upcast on host.
"""

import os
import sys

sys.path.insert(0, "/opt/trn_rl_repo")

import numpy as np

import concourse.bacc as bacc
import concourse.bass as bass
import concourse.mybir as mybir
from concourse.bass_utils import run_bass_kernel_spmd
from concourse.tile import TileContext

B, L, D = 4, 8192, 512
NCORES = 8
EH = D // 2          # output channels per core
NET = EH // 128      # e-tiles per core (2)
NDC = D // 128       # contraction chunks (4)
NSUB = 512           # one fp32 PSUM bank of tokens (matmul N limit)
PSEG = 1024          # tokens per PSUM tile / ACT instruction
# Scan units: one STT + scan per unit; matmul/ACT work in <=1024 chunks.
# The x feed delivers ~0.37 MB/us on one HWDGE ring (~HBM roofline share)
# vs the PE's 0.30 MB/us consumption, so the ramp below is feed-matched:
# starting the PE earlier or splitting x across DMA rings was measured to
# starve the early segments (the SDMA engines and HBM are shared).
# Last unit kept small: the post-last-matmul critical chain is
# ACT->bneg->scan->store on the final unit's tokens. [2304, 256] has the
# same DVE cycle total as [2048, 512] (the scan cost is linear) but a
# ~1.3us shorter exposed tail.
SEGS = [512, 1024, 2048, 2048, 2304, 256]
assert sum(SEGS) == L
MAXSEG = max(SEGS)

FP32 = mybir.dt.float32
F16 = mybir.dt.float16
_last_results = None

# Sized so the warm-up burst ends right as x segment 0 lands (~12.3us):
# any PE-idle gap between warm-up and the real stream restarts the HAM
# busy-window and the first ~5us of real matmuls run at half clock.
N_WARMUP_MM = 40
# Measured (twice): ANY GpSimd Q7 tensor op running concurrently with DVE
# work inflates DVE op durations ~20-30% (SBUF port contention from the
# software engine), a strict net loss since DVE is the pacer. All
# elementwise work therefore stays on DVE; GpSimd only does DMA.


def build_nc() -> bass.Bass:
    # Bacc (not plain Bass): its compile() runs move_matmul_waits_to_ldweights
    # and generate_event_semaphores, which split multi-sem waits to satisfy the
    # TRN2 per-instruction wait-slot limits walrus enforces.
    nc = bacc.Bacc()

    xr = nc.dram_tensor("xr", [128, NDC, L], F16, kind="ExternalInput")
    wg = nc.dram_tensor("wg", [128, NDC, EH], F16, kind="ExternalInput")
    wc = nc.dram_tensor("wc", [128, NDC, EH], F16, kind="ExternalInput")
    # bias packed [128, 4]: cols 0..1 = -bg per e-tile, 2..3 = bc per e-tile
    bias = nc.dram_tensor("bias", [128, 2 * NET], FP32, kind="ExternalInput")
    h = nc.dram_tensor("h", [NET, 128, L], F16, kind="ExternalOutput")
    h_pel = h.rearrange("e p l -> p e l")

    op = mybir.AluOpType
    act = mybir.ActivationFunctionType

    with TileContext(nc) as tc:
        with (
            tc.tile_pool(name="consts", bufs=1) as consts,
            tc.tile_pool(name="xpool", bufs=3) as xpool,
            tc.tile_pool(name="work", bufs=4) as work,
            tc.tile_pool(name="mpool", bufs=1) as mpool,
            tc.tile_pool(name="hpool", bufs=3) as hpool,
            tc.tile_pool(name="psum", bufs=2, space="PSUM") as psum,
        ):
            # PE warm-up: zero a dummy tile, then issue back-to-back matmuls
            # on it while the first weight/x DMAs are still in flight, so
            # PE_HAM releases the 4/8 cold clock gate before the real stream.
            dummy = consts.tile([128, 128], F16)
            nc.gpsimd.memset(dummy, 0.0)
            warm_ps = psum.tile([128, PSEG], FP32, tag="pg", name="warm")
            for _ in range(N_WARMUP_MM):
                nc.tensor.matmul(
                    warm_ps[:, 0:128], dummy, dummy, start=True, stop=True
                )

            # Sync HWDGE queue order: wg chunk 0 -> x seg 0 -> wg rest -> wc
            # -> x seg 1 -> ...  The very first matmul needs only wg[dc=0]
            # and the head of x, so those ship first. Bias rides SWDGE.
            wg_sb = consts.tile([128, NDC, EH], F16)
            wc_sb = consts.tile([128, NDC, EH], F16)
            nc.sync.dma_start(wg_sb[:, 0:1, :], wg[:, 0:1, :])
            x0_sb = xpool.tile([128, NDC, MAXSEG], F16, tag="x", name="x_0")[
                :, :, : SEGS[0]
            ]
            nc.sync.dma_start(x0_sb, xr[:, :, 0 : SEGS[0]])
            nc.sync.dma_start(wg_sb[:, 1:NDC, :], wg[:, 1:NDC, :])
            nc.sync.dma_start(wc_sb, wc[:])

            bias_sb = consts.tile([128, 2 * NET], FP32)
            nc.gpsimd.dma_start(bias_sb, bias[:])

            carry = [None] * NET  # [128, 1] AP of the previous h column
            pending_store = None  # (l0, lt, h2) delayed one unit so the
            # gpsimd queue never head-of-line blocks its bn ops on a scan

            l0 = 0
            for t, lt in enumerate(SEGS):
                if t == 0:
                    x_sb = x0_sb
                else:
                    x_sb = xpool.tile(
                        [128, NDC, MAXSEG], F16, tag="x", name=f"x_{t}"
                    )[:, :, :lt]
                    nc.sync.dma_start(x_sb, xr[:, :, l0 : l0 + lt])

                h2 = hpool.tile([128, NET, MAXSEG], F16, tag="h", name=f"h_{t}")
                for et in range(NET):
                    esl = slice(et * 128, (et + 1) * 128)
                    a_t = work.tile(
                        [128, MAXSEG], F16, tag=f"a{et}", name=f"a{et}_{t}"
                    )[:, :lt]
                    c_t = work.tile(
                        [128, MAXSEG], F16, tag=f"c{et}", name=f"c{et}_{t}"
                    )[:, :lt]
                    # 1024-token PSUM passes fill the scan unit. Separate
                    # pg/pc tags: a merged 4-bank tile was measured to
                    # serialize the MM stream (+14us on the PE).
                    for p0 in range(0, lt, PSEG):
                        pw = min(PSEG, lt - p0)
                        pg = psum.tile(
                            [128, PSEG], FP32, tag="pg", name=f"pg{et}_{t}_{p0}"
                        )
                        pc = psum.tile(
                            [128, PSEG], FP32, tag="pc", name=f"pc{et}_{t}_{p0}"
                        )
                        for n0 in range(0, pw, NSUB):
                            w = min(NSUB, pw - n0)
                            xsl = slice(p0 + n0, p0 + n0 + w)
                            for dc in range(NDC):
                                nc.tensor.matmul(
                                    pg[:, n0 : n0 + w],
                                    wg_sb[:, dc, esl],
                                    x_sb[:, dc, xsl],
                                    start=(dc == 0),
                                    stop=(dc == NDC - 1),
                                )
                            for dc in range(NDC):
                                nc.tensor.matmul(
                                    pc[:, n0 : n0 + w],
                                    wc_sb[:, dc, esl],
                                    x_sb[:, dc, xsl],
                                    start=(dc == 0),
                                    stop=(dc == NDC - 1),
                                )
                        # a = sigmoid(-(z_g + bg)) = 1 - g ; c = tanh(z_c + bc)
                        nc.scalar.activation(
                            a_t[:, p0 : p0 + pw], pg[:, :pw], act.Sigmoid,
                            bias=bias_sb[:, et : et + 1], scale=-1.0,
                        )
                        nc.scalar.activation(
                            c_t[:, p0 : p0 + pw], pc[:, :pw], act.Tanh,
                            bias=bias_sb[:, NET + et : NET + et + 1], scale=1.0,
                        )
                    # bneg = (a - 1) * c = -g * c. Two DVE ops instead of the
                    # scalar_tensor_tensor: tensor_scalar runs in 4x mode and
                    # tensor_tensor in 2x mode for fp16 (the STT has no fast
                    # uop and is stuck at 1x) — ~25% cheaper despite being
                    # two instructions, and both are same-engine so no extra
                    # cross-engine semaphores.
                    am1 = mpool.tile(
                        [128, MAXSEG], F16, tag=f"m{et}", name=f"m{et}_{t}"
                    )[:, :lt]
                    nc.vector.tensor_scalar_sub(am1, a_t, 1.0)
                    bn_t = work.tile(
                        [128, MAXSEG], F16, tag=f"b{et}", name=f"b{et}_{t}"
                    )[:, :lt]
                    nc.vector.tensor_mul(bn_t, am1, c_t)
                    # h = a * h_prev - bneg  (fp32 state in HW, fp16 storage)
                    init = 0.0 if carry[et] is None else carry[et]
                    nc.vector.tensor_tensor_scan(
                        h2[:, et, :lt], a_t, bn_t, init, op.mult, op.subtract
                    )
                    carry[et] = h2[:, et, lt - 1 : lt]
                # One store per unit covering both e-tiles (SWDGE), emitted
                # one unit late.
                if pending_store is not None:
                    pl0, plt, ph2 = pending_store
                    nc.gpsimd.dma_start(
                        h_pel[:, :, pl0 : pl0 + plt], ph2[:, :, :plt]
                    )
                pending_store = (l0, lt, h2)
                l0 += lt
            # Final store rides the idle sync HWDGE ring (lower fixed cost
            # than SWDGE) since every x load has long since drained.
            pl0, plt, ph2 = pending_store
            nc.sync.dma_start(h_pel[:, :, pl0 : pl0 + plt], ph2[:, :, :plt])
    return nc


def _in_maps(x, Wg, bg, Wc, bc):
    maps = []
    xr = {}
    for c in range(NCORES):
        b, eh = c // 2, c % 2
        e0 = eh * EH
        if b not in xr:
            # [L, D] -> [D, L] -> [dc, p, L] -> [p, dc, L] fp16
            xr[b] = x[b].T.reshape(NDC, 128, L).transpose(1, 0, 2).astype(np.float16)
        bias_pack = np.concatenate(
            [
                (-bg[e0 : e0 + EH]).reshape(NET, 128).T,
                bc[e0 : e0 + EH].reshape(NET, 128).T,
            ],
            axis=1,
        ).astype(np.float32)
        maps.append(
            {
                "xr": xr[b],
                "wg": Wg[e0 : e0 + EH].T.reshape(NDC, 128, EH)
                .transpose(1, 0, 2).astype(np.float16),
                "wc": Wc[e0 : e0 + EH].T.reshape(NDC, 128, EH)
                .transpose(1, 0, 2).astype(np.float16),
                "bias": np.ascontiguousarray(bias_pack),
            }
        )
    return maps


def kernel(x, Wg, bg, Wc, bc):
    global _last_results
    x = np.asarray(x, dtype=np.float32)
    Wg = np.asarray(Wg, dtype=np.float32)
    bg = np.asarray(bg, dtype=np.float32)
    Wc = np.asarray(Wc, dtype=np.float32)
    bc = np.asarray(bc, dtype=np.float32)

    nc = build_nc()
    if not nc.is_finalized():
        nc.finalize()
    res = run_bass_kernel_spmd(
        nc,
        _in_maps(x, Wg, bg, Wc, bc),
        list(range(NCORES)),
        tmpdir=os.environ.get("KERNEL_TMPDIR"),
    )
    _last_results = res

    out = np.empty((B, L, D), dtype=np.float32)
    for b in range(B):
        hb = np.concatenate(
            [
                res.results[2 * b]["h"].reshape(EH, L),
                res.results[2 * b + 1]["h"].reshape(EH, L),
            ],
            axis=0,
        ).astype(np.float32)
        out[b] = hb.T
    return out


# revision 48
# speedup vs baseline: 1.0320x; 1.0320x over previous
"""MinGRU kernel for Trainium2 (8 NeuronCores, Bass/Tile) — final.

Measured 82.3-85.1 us exec across runs (baseline 90.5us), rel err 1.38e-3.

Reference computation (B=4, L=8192, D=512, fp32):
    gates = sigmoid(x @ Wg.T + bg)
    cands = tanh(x @ Wc.T + bc)
    h_t   = (1 - g_t) * h_{t-1} + g_t * c_t   (scan along L, h_0 = 0)

Sharding: core c -> (batch b = c//2, channel half = c%2). Each core computes
its batch's full L range for 256 of the 512 output channels; the scan along L
is per (b, channel) so no cross-core communication is needed.

Measured engine budget per core (v1 trace): PE 54.6us (fixed roofline:
131072 PE rows), DVE scan 2.09ns/elem + bneg STT 1.04ns/elem over 16384
elems/lane = 51us payload — DVE is the co-bottleneck, so the design
minimizes DVE instruction count and keeps the dependency graph
single-chain (v2's DVE<->GpSimd ping-pong doubled semaphore costs and
regressed; GpSimd tensor ops run at ~2ns/elem + ~570ns/op and are not
worth it).

v3 vs v1 (90.5us):
  * x and W cast to fp16 on the host: input DMA halves (16.8 -> 8.9 MB/core);
    the x feed (33us queue-wall) ducks well under the PE roofline.
  * Scan units of 2048 tokens: one STT + one scan per (unit, e-tile) with
    matmul/ACT filling the unit in 1024-token halves ([128,1024] fp32 PSUM
    tiles = 2 banks, tags g/c x bufs 2 = 8 banks, still double-buffered).
    Fewer DVE ops -> less fixed overhead and fewer semaphores.
  * Activations read the full 1024-token PSUM tile in one instruction.
  * 26 warm-up matmuls on a zeroed dummy tile while the first weight/x DMAs
    fly: PE_HAM releases the 4/8 cold clock gate before real matmuls start,
    and the PE is never idle long enough to re-throttle.
  * wg ships in two pieces (dc chunk 0 first) so the first real matmul only
    waits for 64KB of weights plus the first x segment.
  * Segment ramp [512, 1024, 1536, 2048, ...] matched to the x queue's
    ~0.37 MB/us delivery rate so the PE rarely outruns the feed.
  * -bg negated on the host; h stored fp16 [2, 128, L] and upcast on host.
"""

import os
import sys

sys.path.insert(0, "/opt/trn_rl_repo")

import numpy as np

import concourse.bacc as bacc
import concourse.bass as bass
import concourse.mybir as mybir
from concourse.bass_utils import run_bass_kernel_spmd
from concourse.tile import TileContext

B, L, D = 4, 8192, 512
NCORES = 8
EH = D // 2          # output channels per core
NET = EH // 128      # e-tiles per core (2)
NDC = D // 128       # contraction chunks (4)
NSUB = 512           # one fp32 PSUM bank of tokens (matmul N limit)
PSEG = 1024          # tokens per PSUM tile / ACT instruction
# Scan units: one STT + scan per unit; matmul/ACT work in <=1024 chunks.
# The x feed delivers ~0.37 MB/us on one HWDGE ring (~HBM roofline share)
# vs the PE's 0.30 MB/us consumption, so the ramp below is feed-matched:
# starting the PE earlier or splitting x across DMA rings was measured to
# starve the early segments (the SDMA engines and HBM are shared).
SEGS = [512, 1024, 2048, 2048, 2048, 512]
assert sum(SEGS) == L
MAXSEG = max(SEGS)

FP32 = mybir.dt.float32
F16 = mybir.dt.float16
_last_results = None

# Sized so the warm-up burst ends right as x segment 0 lands (~12.3us):
# any PE-idle gap between warm-up and the real stream restarts the HAM
# busy-window and the first ~5us of real matmuls run at half clock.
N_WARMUP_MM = 40
# Measured (twice): ANY GpSimd Q7 tensor op running concurrently with DVE
# work inflates DVE op durations ~20-30% (SBUF port contention from the
# software engine), a strict net loss since DVE is the pacer. All
# elementwise work therefore stays on DVE; GpSimd only does DMA.


def build_nc() -> bass.Bass:
    # Bacc (not plain Bass): its compile() runs move_matmul_waits_to_ldweights
    # and generate_event_semaphores, which split multi-sem waits to satisfy the
    # TRN2 per-instruction wait-slot limits walrus enforces.
    nc = bacc.Bacc()

    xr = nc.dram_tensor("xr", [128, NDC, L], F16, kind="ExternalInput")
    wg = nc.dram_tensor("wg", [128, NDC, EH], F16, kind="ExternalInput")
    wc = nc.dram_tensor("wc", [128, NDC, EH], F16, kind="ExternalInput")
    # bias packed [128, 4]: cols 0..1 = -bg per e-tile, 2..3 = bc per e-tile
    bias = nc.dram_tensor("bias", [128, 2 * NET], FP32, kind="ExternalInput")
    h = nc.dram_tensor("h", [NET, 128, L], F16, kind="ExternalOutput")
    h_pel = h.rearrange("e p l -> p e l")

    op = mybir.AluOpType
    act = mybir.ActivationFunctionType

    with TileContext(nc) as tc:
        with (
            tc.tile_pool(name="consts", bufs=1) as consts,
            tc.tile_pool(name="xpool", bufs=3) as xpool,
            tc.tile_pool(name="work", bufs=4) as work,
            tc.tile_pool(name="mpool", bufs=1) as mpool,
            tc.tile_pool(name="hpool", bufs=3) as hpool,
            tc.tile_pool(name="psum", bufs=2, space="PSUM") as psum,
        ):
            # PE warm-up: zero a dummy tile, then issue back-to-back matmuls
            # on it while the first weight/x DMAs are still in flight, so
            # PE_HAM releases the 4/8 cold clock gate before the real stream.
            dummy = consts.tile([128, 128], F16)
            nc.gpsimd.memset(dummy, 0.0)
            warm_ps = psum.tile([128, PSEG], FP32, tag="pg", name="warm")
            for _ in range(N_WARMUP_MM):
                nc.tensor.matmul(
                    warm_ps[:, 0:128], dummy, dummy, start=True, stop=True
                )

            # Sync HWDGE queue order: wg chunk 0 -> x seg 0 -> wg rest -> wc
            # -> x seg 1 -> ...  The very first matmul needs only wg[dc=0]
            # and the head of x, so those ship first. Bias rides SWDGE.
            wg_sb = consts.tile([128, NDC, EH], F16)
            wc_sb = consts.tile([128, NDC, EH], F16)
            nc.sync.dma_start(wg_sb[:, 0:1, :], wg[:, 0:1, :])
            x0_sb = xpool.tile([128, NDC, MAXSEG], F16, tag="x", name="x_0")[
                :, :, : SEGS[0]
            ]
            nc.sync.dma_start(x0_sb, xr[:, :, 0 : SEGS[0]])
            nc.sync.dma_start(wg_sb[:, 1:NDC, :], wg[:, 1:NDC, :])
            nc.sync.dma_start(wc_sb, wc[:])

            bias_sb = consts.tile([128, 2 * NET], FP32)
            nc.gpsimd.dma_start(bias_sb, bias[:])

            carry = [None] * NET  # [128, 1] AP of the previous h column
            pending_store = None  # (l0, lt, h2) delayed one unit so the
            # gpsimd queue never head-of-line blocks its bn ops on a scan

            l0 = 0
            for t, lt in enumerate(SEGS):
                if t == 0:
                    x_sb = x0_sb
                else:
                    x_sb = xpool.tile(
                        [128, NDC, MAXSEG], F16, tag="x", name=f"x_{t}"
                    )[:, :, :lt]
                    nc.sync.dma_start(x_sb, xr[:, :, l0 : l0 + lt])

                h2 = hpool.tile([128, NET, MAXSEG], F16, tag="h", name=f"h_{t}")
                for et in range(NET):
                    esl = slice(et * 128, (et + 1) * 128)
                    a_t = work.tile(
                        [128, MAXSEG], F16, tag=f"a{et}", name=f"a{et}_{t}"
                    )[:, :lt]
                    c_t = work.tile(
                        [128, MAXSEG], F16, tag=f"c{et}", name=f"c{et}_{t}"
                    )[:, :lt]
                    # 1024-token PSUM passes fill the scan unit. Separate
                    # pg/pc tags: a merged 4-bank tile was measured to
                    # serialize the MM stream (+14us on the PE).
                    for p0 in range(0, lt, PSEG):
                        pw = min(PSEG, lt - p0)
                        pg = psum.tile(
                            [128, PSEG], FP32, tag="pg", name=f"pg{et}_{t}_{p0}"
                        )
                        pc = psum.tile(
                            [128, PSEG], FP32, tag="pc", name=f"pc{et}_{t}_{p0}"
                        )
                        for n0 in range(0, pw, NSUB):
                            w = min(NSUB, pw - n0)
                            xsl = slice(p0 + n0, p0 + n0 + w)
                            for dc in range(NDC):
                                nc.tensor.matmul(
                                    pg[:, n0 : n0 + w],
                                    wg_sb[:, dc, esl],
                                    x_sb[:, dc, xsl],
                                    start=(dc == 0),
                                    stop=(dc == NDC - 1),
                                )
                            for dc in range(NDC):
                                nc.tensor.matmul(
                                    pc[:, n0 : n0 + w],
                                    wc_sb[:, dc, esl],
                                    x_sb[:, dc, xsl],
                                    start=(dc == 0),
                                    stop=(dc == NDC - 1),
                                )
                        # a = sigmoid(-(z_g + bg)) = 1 - g ; c = tanh(z_c + bc)
                        nc.scalar.activation(
                            a_t[:, p0 : p0 + pw], pg[:, :pw], act.Sigmoid,
                            bias=bias_sb[:, et : et + 1], scale=-1.0,
                        )
                        nc.scalar.activation(
                            c_t[:, p0 : p0 + pw], pc[:, :pw], act.Tanh,
                            bias=bias_sb[:, NET + et : NET + et + 1], scale=1.0,
                        )
                    # bneg = (a - 1) * c = -g * c. Two DVE ops instead of the
                    # scalar_tensor_tensor: tensor_scalar runs in 4x mode and
                    # tensor_tensor in 2x mode for fp16 (the STT has no fast
                    # uop and is stuck at 1x) — ~25% cheaper despite being
                    # two instructions, and both are same-engine so no extra
                    # cross-engine semaphores.
                    am1 = mpool.tile(
                        [128, MAXSEG], F16, tag=f"m{et}", name=f"m{et}_{t}"
                    )[:, :lt]
                    nc.vector.tensor_scalar_sub(am1, a_t, 1.0)
                    bn_t = work.tile(
                        [128, MAXSEG], F16, tag=f"b{et}", name=f"b{et}_{t}"
                    )[:, :lt]
                    nc.vector.tensor_mul(bn_t, am1, c_t)
                    # h = a * h_prev - bneg  (fp32 state in HW, fp16 storage)
                    init = 0.0 if carry[et] is None else carry[et]
                    nc.vector.tensor_tensor_scan(
                        h2[:, et, :lt], a_t, bn_t, init, op.mult, op.subtract
                    )
                    carry[et] = h2[:, et, lt - 1 : lt]
                # One store per unit covering both e-tiles (SWDGE), emitted
                # one unit late.
                if pending_store is not None:
                    pl0, plt, ph2 = pending_store
                    nc.gpsimd.dma_start(
                        h_pel[:, :, pl0 : pl0 + plt], ph2[:, :, :plt]
                    )
                pending_store = (l0, lt, h2)
                l0 += lt
            # Final store rides the idle sync HWDGE ring (lower fixed cost
            # than SWDGE) since every x load has long since drained.
            pl0, plt, ph2 = pending_store
            nc.sync.dma_start(h_pel[:, :, pl0 : pl0 + plt], ph2[:, :, :plt])
    return nc


def _in_maps(x, Wg, bg, Wc, bc):
    maps = []
    xr = {}
    for c in range(NCORES):
        b, eh = c // 2, c % 2
        e0 = eh * EH
        if b not in xr:
            # [L, D] -> [D, L] -> [dc, p, L] -> [p, dc, L] fp16
            xr[b] = x[b].T.reshape(NDC, 128, L).transpose(1, 0, 2).astype(np.float16)
        bias_pack = np.concatenate(
            [
                (-bg[e0 : e0 + EH]).reshape(NET, 128).T,
                bc[e0 : e0 + EH].reshape(NET, 128).T,
            ],
            axis=1,
        ).astype(np.float32)
        maps.append(
            {
                "xr": xr[b],
                "wg": Wg[e0 : e0 + EH].T.reshape(NDC, 128, EH)
                .transpose(1, 0, 2).astype(np.float16),
                "wc": Wc[e0 : e0 + EH].T.reshape(NDC, 128, EH)
                .transpose(1, 0, 2).astype(np.float16),
                "bias": np.ascontiguousarray(bias_pack),
            }
        )
    return maps


def kernel(x, Wg, bg, Wc, bc):
    global _last_results
    x = np.asarray(x, dtype=np.float32)
    Wg = np.asarray(Wg, dtype=np.float32)
    bg = np.asarray(bg, dtype=np.float32)
    Wc = np.asarray(Wc, dtype=np.float32)
    bc = np.asarray(bc, dtype=np.float32)

    nc = build_nc()
    if not nc.is_finalized():
        nc.finalize()
    res = run_bass_kernel_spmd(
        nc,
        _in_maps(x, Wg, bg, Wc, bc),
        list(range(NCORES)),
        tmpdir=os.environ.get("KERNEL_TMPDIR"),
    )
    _last_results = res

    out = np.empty((B, L, D), dtype=np.float32)
    for b in range(B):
        hb = np.concatenate(
            [
                res.results[2 * b]["h"].reshape(EH, L),
                res.results[2 * b + 1]["h"].reshape(EH, L),
            ],
            axis=0,
        ).astype(np.float32)
        out[b] = hb.T
    return out


# revision 52
# speedup vs baseline: 1.0464x; 1.0140x over previous
"""MinGRU kernel for Trainium2 (8 NeuronCores, Bass/Tile) — final.

Measured 82.3-85.1 us exec across runs (baseline 90.5us), rel err 1.38e-3.

Reference computation (B=4, L=8192, D=512, fp32):
    gates = sigmoid(x @ Wg.T + bg)
    cands = tanh(x @ Wc.T + bc)
    h_t   = (1 - g_t) * h_{t-1} + g_t * c_t   (scan along L, h_0 = 0)

Sharding: core c -> (batch b = c//2, channel half = c%2). Each core computes
its batch's full L range for 256 of the 512 output channels; the scan along L
is per (b, channel) so no cross-core communication is needed.

Measured engine budget per core (v1 trace): PE 54.6us (fixed roofline:
131072 PE rows), DVE scan 2.09ns/elem + bneg STT 1.04ns/elem over 16384
elems/lane = 51us payload — DVE is the co-bottleneck, so the design
minimizes DVE instruction count and keeps the dependency graph
single-chain (v2's DVE<->GpSimd ping-pong doubled semaphore costs and
regressed; GpSimd tensor ops run at ~2ns/elem + ~570ns/op and are not
worth it).

v3 vs v1 (90.5us):
  * x and W cast to fp16 on the host: input DMA halves (16.8 -> 8.9 MB/core);
    the x feed (33us queue-wall) ducks well under the PE roofline.
  * Scan units of 2048 tokens: one STT + one scan per (unit, e-tile) with
    matmul/ACT filling the unit in 1024-token halves ([128,1024] fp32 PSUM
    tiles = 2 banks, tags g/c x bufs 2 = 8 banks, still double-buffered).
    Fewer DVE ops -> less fixed overhead and fewer semaphores.
  * Activations read the full 1024-token PSUM tile in one instruction.
  * 26 warm-up matmuls on a zeroed dummy tile while the first weight/x DMAs
    fly: PE_HAM releases the 4/8 cold clock gate before real matmuls start,
    and the PE is never idle long enough to re-throttle.
  * wg ships in two pieces (dc chunk 0 first) so the first real matmul only
    waits for 64KB of weights plus the first x segment.
  * Segment ramp [512, 1024, 1536, 2048, ...] matched to the x queue's
    ~0.37 MB/us delivery rate so the PE rarely outruns the feed.
  * -bg negated on the host; h stored fp16 [2, 128, L] and upcast on host.
"""

import os
import sys

sys.path.insert(0, "/opt/trn_rl_repo")

import numpy as np

import concourse.bacc as bacc
import concourse.bass as bass
import concourse.mybir as mybir
from concourse.bass_utils import run_bass_kernel_spmd
from concourse.tile import TileContext

B, L, D = 4, 8192, 512
NCORES = 8
EH = D // 2          # output channels per core
NET = EH // 128      # e-tiles per core (2)
NDC = D // 128       # contraction chunks (4)
NSUB = 512           # one fp32 PSUM bank of tokens (matmul N limit)
PSEG = 1024          # tokens per PSUM tile / ACT instruction
# Scan units: one STT + scan per unit; matmul/ACT work in <=1024 chunks.
# The x feed delivers ~0.37 MB/us on one HWDGE ring (~HBM roofline share)
# vs the PE's 0.30 MB/us consumption, so the ramp below is feed-matched:
# starting the PE earlier or splitting x across DMA rings was measured to
# starve the early segments (the SDMA engines and HBM are shared).
SEGS = [512, 1024, 2048, 2048, 2048, 512]
assert sum(SEGS) == L
MAXSEG = max(SEGS)

FP32 = mybir.dt.float32
F16 = mybir.dt.float16
_last_results = None

# Sized so the warm-up burst ends right as x segment 0 lands (~12.3us):
# any PE-idle gap between warm-up and the real stream restarts the HAM
# busy-window and the first ~5us of real matmuls run at half clock.
N_WARMUP_MM = 40
# Measured (twice): ANY GpSimd Q7 tensor op running concurrently with DVE
# work inflates DVE op durations ~20-30% (SBUF port contention from the
# software engine), a strict net loss since DVE is the pacer. All
# elementwise work therefore stays on DVE; GpSimd only does DMA.


def build_nc() -> bass.Bass:
    # Bacc (not plain Bass): its compile() runs move_matmul_waits_to_ldweights
    # and generate_event_semaphores, which split multi-sem waits to satisfy the
    # TRN2 per-instruction wait-slot limits walrus enforces.
    nc = bacc.Bacc()

    xr = nc.dram_tensor("xr", [128, NDC, L], F16, kind="ExternalInput")
    wg = nc.dram_tensor("wg", [128, NDC, EH], F16, kind="ExternalInput")
    wc = nc.dram_tensor("wc", [128, NDC, EH], F16, kind="ExternalInput")
    # bias packed [128, 4]: cols 0..1 = -bg per e-tile, 2..3 = bc per e-tile
    bias = nc.dram_tensor("bias", [128, 2 * NET], FP32, kind="ExternalInput")
    h = nc.dram_tensor("h", [NET, 128, L], F16, kind="ExternalOutput")
    h_pel = h.rearrange("e p l -> p e l")

    op = mybir.AluOpType
    act = mybir.ActivationFunctionType

    with TileContext(nc) as tc:
        with (
            tc.tile_pool(name="consts", bufs=1) as consts,
            tc.tile_pool(name="xpool", bufs=3) as xpool,
            tc.tile_pool(name="work", bufs=4) as work,
            tc.tile_pool(name="mpool", bufs=1) as mpool,
            tc.tile_pool(name="hpool", bufs=3) as hpool,
            tc.tile_pool(name="psum", bufs=2, space="PSUM") as psum,
        ):
            # PE warm-up: zero a dummy tile, then issue back-to-back matmuls
            # on it while the first weight/x DMAs are still in flight, so
            # PE_HAM releases the 4/8 cold clock gate before the real stream.
            dummy = consts.tile([128, 128], F16)
            nc.vector.memset(dummy, 0.0)
            warm_ps = psum.tile([128, PSEG], FP32, tag="pg", name="warm")
            for _ in range(N_WARMUP_MM):
                nc.tensor.matmul(
                    warm_ps[:, 0:128], dummy, dummy, start=True, stop=True
                )

            # Everything rides the sync HWDGE ring (no SWDGE at all: an idle
            # GpSimd skips its expensive dge_drain in the epilogue). Queue
            # order: wg chunk 0 -> x seg 0 -> bias -> wg rest -> wc -> all
            # remaining x segs (their dispatches self-pace on the x-tile ring
            # WAR waits) -> h stores (emitted after, so they can never
            # head-of-line block an x load).
            wg_sb = consts.tile([128, NDC, EH], F16)
            wc_sb = consts.tile([128, NDC, EH], F16)
            nc.sync.dma_start(wg_sb[:, 0:1, :], wg[:, 0:1, :])
            x_tiles = [
                xpool.tile([128, NDC, MAXSEG], F16, tag="x", name=f"x_{t}")[
                    :, :, :lt
                ]
                for t, lt in enumerate(SEGS)
            ]
            nc.sync.dma_start(x_tiles[0], xr[:, :, 0 : SEGS[0]])
            bias_sb = consts.tile([128, 2 * NET], FP32)
            nc.sync.dma_start(bias_sb, bias[:])
            nc.sync.dma_start(wg_sb[:, 1:NDC, :], wg[:, 1:NDC, :])
            nc.sync.dma_start(wc_sb, wc[:])
            l0 = SEGS[0]
            for t, lt in enumerate(SEGS[1:], start=1):
                nc.sync.dma_start(x_tiles[t], xr[:, :, l0 : l0 + lt])
                l0 += lt

            carry = [None] * NET  # [128, 1] AP of the previous h column
            pending_store = None  # (l0, lt, h2) delayed one unit so the
            # gpsimd queue never head-of-line blocks its bn ops on a scan

            l0 = 0
            for t, lt in enumerate(SEGS):
                x_sb = x_tiles[t]
                h2 = hpool.tile([128, NET, MAXSEG], F16, tag="h", name=f"h_{t}")
                for et in range(NET):
                    esl = slice(et * 128, (et + 1) * 128)
                    a_t = work.tile(
                        [128, MAXSEG], F16, tag=f"a{et}", name=f"a{et}_{t}"
                    )[:, :lt]
                    c_t = work.tile(
                        [128, MAXSEG], F16, tag=f"c{et}", name=f"c{et}_{t}"
                    )[:, :lt]
                    # 1024-token PSUM passes fill the scan unit. Separate
                    # pg/pc tags: a merged 4-bank tile was measured to
                    # serialize the MM stream (+14us on the PE).
                    for p0 in range(0, lt, PSEG):
                        pw = min(PSEG, lt - p0)
                        pg = psum.tile(
                            [128, PSEG], FP32, tag="pg", name=f"pg{et}_{t}_{p0}"
                        )
                        pc = psum.tile(
                            [128, PSEG], FP32, tag="pc", name=f"pc{et}_{t}_{p0}"
                        )
                        for n0 in range(0, pw, NSUB):
                            w = min(NSUB, pw - n0)
                            xsl = slice(p0 + n0, p0 + n0 + w)
                            for dc in range(NDC):
                                nc.tensor.matmul(
                                    pg[:, n0 : n0 + w],
                                    wg_sb[:, dc, esl],
                                    x_sb[:, dc, xsl],
                                    start=(dc == 0),
                                    stop=(dc == NDC - 1),
                                )
                            for dc in range(NDC):
                                nc.tensor.matmul(
                                    pc[:, n0 : n0 + w],
                                    wc_sb[:, dc, esl],
                                    x_sb[:, dc, xsl],
                                    start=(dc == 0),
                                    stop=(dc == NDC - 1),
                                )
                        # a = sigmoid(-(z_g + bg)) = 1 - g ; c = tanh(z_c + bc)
                        nc.scalar.activation(
                            a_t[:, p0 : p0 + pw], pg[:, :pw], act.Sigmoid,
                            bias=bias_sb[:, et : et + 1], scale=-1.0,
                        )
                        nc.scalar.activation(
                            c_t[:, p0 : p0 + pw], pc[:, :pw], act.Tanh,
                            bias=bias_sb[:, NET + et : NET + et + 1], scale=1.0,
                        )
                    # bneg = (a - 1) * c = -g * c. Two DVE ops instead of the
                    # scalar_tensor_tensor: tensor_scalar runs in 4x mode and
                    # tensor_tensor in 2x mode for fp16 (the STT has no fast
                    # uop and is stuck at 1x) — ~25% cheaper despite being
                    # two instructions, and both are same-engine so no extra
                    # cross-engine semaphores.
                    am1 = mpool.tile(
                        [128, MAXSEG], F16, tag=f"m{et}", name=f"m{et}_{t}"
                    )[:, :lt]
                    nc.vector.tensor_scalar_sub(am1, a_t, 1.0)
                    bn_t = work.tile(
                        [128, MAXSEG], F16, tag=f"b{et}", name=f"b{et}_{t}"
                    )[:, :lt]
                    nc.vector.tensor_mul(bn_t, am1, c_t)
                    # h = a * h_prev - bneg  (fp32 state in HW, fp16 storage)
                    init = 0.0 if carry[et] is None else carry[et]
                    nc.vector.tensor_tensor_scan(
                        h2[:, et, :lt], a_t, bn_t, init, op.mult, op.subtract
                    )
                    carry[et] = h2[:, et, lt - 1 : lt]
                # One store per unit covering both e-tiles, emitted one unit
                # late (sync ring; all x dispatches precede these in program
                # order so stores cannot delay the feed).
                if pending_store is not None:
                    pl0, plt, ph2 = pending_store
                    nc.sync.dma_start(
                        h_pel[:, :, pl0 : pl0 + plt], ph2[:, :, :plt]
                    )
                pending_store = (l0, lt, h2)
                l0 += lt
            pl0, plt, ph2 = pending_store
            nc.sync.dma_start(h_pel[:, :, pl0 : pl0 + plt], ph2[:, :, :plt])
    return nc


def _in_maps(x, Wg, bg, Wc, bc):
    maps = []
    xr = {}
    for c in range(NCORES):
        b, eh = c // 2, c % 2
        e0 = eh * EH
        if b not in xr:
            # [L, D] -> [D, L] -> [dc, p, L] -> [p, dc, L] fp16
            xr[b] = x[b].T.reshape(NDC, 128, L).transpose(1, 0, 2).astype(np.float16)
        bias_pack = np.concatenate(
            [
                (-bg[e0 : e0 + EH]).reshape(NET, 128).T,
                bc[e0 : e0 + EH].reshape(NET, 128).T,
            ],
            axis=1,
        ).astype(np.float32)
        maps.append(
            {
                "xr": xr[b],
                "wg": Wg[e0 : e0 + EH].T.reshape(NDC, 128, EH)
                .transpose(1, 0, 2).astype(np.float16),
                "wc": Wc[e0 : e0 + EH].T.reshape(NDC, 128, EH)
                .transpose(1, 0, 2).astype(np.float16),
                "bias": np.ascontiguousarray(bias_pack),
            }
        )
    return maps


def kernel(x, Wg, bg, Wc, bc):
    global _last_results
    x = np.asarray(x, dtype=np.float32)
    Wg = np.asarray(Wg, dtype=np.float32)
    bg = np.asarray(bg, dtype=np.float32)
    Wc = np.asarray(Wc, dtype=np.float32)
    bc = np.asarray(bc, dtype=np.float32)

    nc = build_nc()
    if not nc.is_finalized():
        nc.finalize()
    res = run_bass_kernel_spmd(
        nc,
        _in_maps(x, Wg, bg, Wc, bc),
        list(range(NCORES)),
        tmpdir=os.environ.get("KERNEL_TMPDIR"),
    )
    _last_results = res

    out = np.empty((B, L, D), dtype=np.float32)
    for b in range(B):
        hb = np.concatenate(
            [
                res.results[2 * b]["h"].reshape(EH, L),
                res.results[2 * b + 1]["h"].reshape(EH, L),
            ],
            axis=0,
        ).astype(np.float32)
        out[b] = hb.T
    return out
